# revision 1
# baseline (speedup 1.0000x reference)
"""Trainium2 Bass kernel for EnhancedHeteroGNN patent-branch forward.

Only the patent branch feeds the returned logits (the author/SAGE branch is
dead code in the reference), so the computation is:

    xp0 = LN(x_patent) @ pl_W + pl_b
    for layer in (g1, g2):
        T = [xp @ gW | es | ed]          # es/ed folded into the projection
        (all-gather T across 8 cores)
        agg[d] = sum_e exp(lrelu(es[s]+ed[d])) * xp'[s] / den[d]   (4 heads)
        xp = LN(relu(agg + g_b)) * n_w + n_b + xp
    out = relu(xp @ c1_W + c1_b) @ c2_W + c2_b

Sharding: destination-partitioned edges (dst node block of 12500 rows per
core), self-loops included; softmax computed without max subtraction (it is
shift-invariant and the logits here are < 0.5 in magnitude).

Segment softmax+sum is done per 128-dst block with one-hot matmuls
accumulated in PSUM; the denominator rides along as 4 extra matmul columns.
"""

import os

import numpy as np

DBG_STAGE = int(os.environ.get("GNN_DBG_STAGE", "3"))
DBG_BLOCKS = int(os.environ.get("GNN_DBG_BLOCKS", "0"))
DBG_B0 = int(os.environ.get("GNN_DBG_B0", "0"))
DBG_DUMP = int(os.environ.get("GNN_DBG_DUMP", "0"))

N_NODES = 100000
F_IN = 256
HID = 128
NHEAD = 4
CH = HID // NHEAD  # 32
N_CORES = 8
NPC = N_NODES // N_CORES  # 12500 nodes per core
P = 128
N_BLOCKS = (NPC + P - 1) // P  # 98 (97 full + one of 84)
TCOLS = HID + 2 * NHEAD  # 136: [xp' (128) | es (4) | ed (4)]
PAD_DLOC = 999.0


def _prep_edges(ei_cites: np.ndarray):
    """dst-partition edges (plus self loops) into per-core per-block chunks.

    Returns (idx_maps, kb) where idx_maps[c] is an int32 [128, 3*K] array
    (per block b a [128, 3*kb[b]] segment laid out [src | dst | dloc_f32]),
    and kb[b] is the common (max-over-cores) 128-edge chunk count of block b.
    """
    src = np.concatenate([ei_cites[0], np.arange(N_NODES, dtype=np.int64)])
    dst = np.concatenate([ei_cites[1], np.arange(N_NODES, dtype=np.int64)])
    core = dst // NPC

    per_core = []
    cnts = np.zeros((N_CORES, N_BLOCKS), dtype=np.int64)
    for c in range(N_CORES):
        m = core == c
        s_c = src[m]
        d_c = dst[m]
        loc = d_c - c * NPC
        blk = loc // P
        order = np.lexsort((s_c, blk))
        s_c, d_c, loc, blk = s_c[order], d_c[order], loc[order], blk[order]
        cnts[c] = np.bincount(blk, minlength=N_BLOCKS)
        per_core.append((s_c, d_c, loc, blk))

    kb = np.maximum(1, (cnts.max(axis=0) + P - 1) // P).astype(np.int64)
    if os.environ.get("GNN_KB_EVEN", "1") == "1":
        kb = kb + (kb % 2)

    idx_maps = []
    for c in range(N_CORES):
        s_c, d_c, loc, blk = per_core[c]
        starts = np.zeros(N_BLOCKS + 1, dtype=np.int64)
        starts[1:] = np.cumsum(cnts[c])
        segs = []
        for b in range(N_BLOCKS):
            n_b = int(cnts[c, b])
            cap = int(kb[b]) * P
            s_b = np.zeros(cap, dtype=np.int32)
            d_b = np.zeros(cap, dtype=np.int32)
            dl_b = np.full(cap, PAD_DLOC, dtype=np.float32)
            sl = slice(starts[b], starts[b + 1])
            s_b[:n_b] = s_c[sl]
            d_b[:n_b] = d_c[sl]
            dl_b[:n_b] = (loc[sl] - b * P).astype(np.float32)
            # edge j -> (partition j%128, column j//128)
            s2 = s_b.reshape(-1, P).T  # [128, kb]
            d2 = d_b.reshape(-1, P).T
            dl2 = dl_b.reshape(-1, P).T.view(np.int32)
            segs.append(np.concatenate([s2, d2, dl2], axis=1))
        idx_maps.append(np.ascontiguousarray(np.concatenate(segs, axis=1)))
    return idx_maps, kb


def _head_fold(W: np.ndarray, a_s: np.ndarray, a_d: np.ndarray) -> np.ndarray:
    """[gW | gW@As | gW@Ad] where As/Ad are the blockdiag head-attention maps."""
    A_s = np.zeros((HID, NHEAD), dtype=np.float32)
    A_d = np.zeros((HID, NHEAD), dtype=np.float32)
    for h in range(NHEAD):
        A_s[h * CH:(h + 1) * CH, h] = a_s[h]
        A_d[h * CH:(h + 1) * CH, h] = a_d[h]
    return np.concatenate([W, W @ A_s, W @ A_d], axis=1).astype(np.float32)


def _build(kb):
    import concourse.bass as bass
    import concourse.mybir as mybir
    import concourse.tile as tile
    from concourse import bacc
    from concourse.masks import make_identity

    f32 = mybir.dt.float32
    i32 = mybir.dt.int32
    K_total = int(kb.sum())
    KMAX = int(kb.max())

    nc = bacc.Bacc("TRN2", num_devices=N_CORES,
                   target_bir_lowering=False,
                   dynamic_dma_scratch_size=int(os.environ.get(
                       "GNN_DDMA_SCRATCH", "16384")))

    x_in = nc.dram_tensor("x", [NPC, F_IN], f32, kind="ExternalInput")
    idx_in = nc.dram_tensor("idx", [P, 3 * K_total], i32, kind="ExternalInput")
    pnw_in = nc.dram_tensor("pn_w", [F_IN], f32, kind="ExternalInput")
    pnb_in = nc.dram_tensor("pn_b", [F_IN], f32, kind="ExternalInput")
    plw_in = nc.dram_tensor("plw", [P, 2, HID], f32, kind="ExternalInput")
    plb_in = nc.dram_tensor("pl_b", [HID], f32, kind="ExternalInput")
    g1_in = nc.dram_tensor("g1ext", [HID, TCOLS], f32, kind="ExternalInput")
    g1b_in = nc.dram_tensor("g1_b", [HID], f32, kind="ExternalInput")
    g2_in = nc.dram_tensor("g2ext", [HID, TCOLS], f32, kind="ExternalInput")
    g2b_in = nc.dram_tensor("g2_b", [HID], f32, kind="ExternalInput")
    n1w_in = nc.dram_tensor("n1_w", [HID], f32, kind="ExternalInput")
    n1b_in = nc.dram_tensor("n1_b", [HID], f32, kind="ExternalInput")
    n3w_in = nc.dram_tensor("n3_w", [HID], f32, kind="ExternalInput")
    n3b_in = nc.dram_tensor("n3_b", [HID], f32, kind="ExternalInput")
    c1w_in = nc.dram_tensor("c1w", [HID, 64], f32, kind="ExternalInput")
    c1b_in = nc.dram_tensor("c1_b", [64], f32, kind="ExternalInput")
    c2w_in = nc.dram_tensor("c2w", [64, 8], f32, kind="ExternalInput")
    c2b_in = nc.dram_tensor("c2_b", [8], f32, kind="ExternalInput")
    out_ext = nc.dram_tensor("out", [NPC, 8], f32, kind="ExternalOutput")
    if DBG_DUMP:
        dbg_t1 = nc.dram_tensor("dbg_t1", [NPC, TCOLS], f32,
                                kind="ExternalOutput")
        dbg_t1f = nc.dram_tensor("dbg_t1f", [2 * P, TCOLS], f32,
                                 kind="ExternalOutput")
        dbg_gat = nc.dram_tensor("dbg_gat", [P, KMAX, TCOLS - NHEAD], f32,
                                 kind="ExternalOutput")
        dbg_ged = nc.dram_tensor("dbg_ged", [P, KMAX, NHEAD], f32,
                                 kind="ExternalOutput")
        dbg_ps = nc.dram_tensor("dbg_ps", [P, TCOLS - NHEAD], f32,
                                kind="ExternalOutput")
        dbg_h1 = nc.dram_tensor("dbg_h1", [P, HID], f32,
                                kind="ExternalOutput")

    xp0_own = nc.dram_tensor("xp0_own", [NPC, HID], f32)
    xp1_own = nc.dram_tensor("xp1_own", [NPC, HID], f32)
    t1_own = nc.dram_tensor("t1_own", [NPC, TCOLS], f32)
    t2_own = nc.dram_tensor("t2_own", [NPC, TCOLS], f32)
    t1_full = nc.dram_tensor("t1_full", [N_NODES, TCOLS], f32, addr_space="Shared")
    t2_full = nc.dram_tensor("t2_full", [N_NODES, TCOLS], f32, addr_space="Shared")

    AOp = mybir.AluOpType
    Act = mybir.ActivationFunctionType

    with tile.TileContext(nc) as tc:
        with tc.tile_pool(name="const", bufs=1) as cpool:
            ident = cpool.tile([P, P], f32)
            make_identity(nc, ident[:])
            iota_i = cpool.tile([P, P], i32)
            nc.gpsimd.iota(iota_i[:], pattern=[[1, P]], base=0,
                           channel_multiplier=0)
            iota_f = cpool.tile([P, P], f32)
            nc.vector.tensor_copy(iota_f[:], iota_i[:])
            eps_t = cpool.tile([P, 1], f32)
            nc.vector.memset(eps_t[:], 1e-5)
            ones_t = cpool.tile([P, NHEAD], f32)
            nc.vector.memset(ones_t[:], 1.0)

            def bcast_load(dram_t, n):
                t = cpool.tile([P, n], f32, tag=f"c_{dram_t.name}")
                nc.sync.dma_start(
                    out=t[:], in_=dram_t[:].unsqueeze(0).to_broadcast([P, n]))
                return t

            pnw_t = bcast_load(pnw_in, F_IN)
            pnb_t = bcast_load(pnb_in, F_IN)
            plb_t = bcast_load(plb_in, HID)
            g1b_t = bcast_load(g1b_in, HID)
            g2b_t = bcast_load(g2b_in, HID)
            n1w_t = bcast_load(n1w_in, HID)
            n1b_t = bcast_load(n1b_in, HID)
            n3w_t = bcast_load(n3w_in, HID)
            n3b_t = bcast_load(n3b_in, HID)
            c1b_t = bcast_load(c1b_in, 64)
            c2b_t = bcast_load(c2b_in, 8)
            plw_t = cpool.tile([P, 2, HID], f32)
            nc.sync.dma_start(out=plw_t[:], in_=plw_in[:, :, :])
            g1_t = cpool.tile([HID, TCOLS], f32)
            nc.sync.dma_start(out=g1_t[:], in_=g1_in[:, :])
            g2_t = cpool.tile([HID, TCOLS], f32)
            nc.sync.dma_start(out=g2_t[:], in_=g2_in[:, :])
            c1w_t = cpool.tile([HID, 64], f32)
            nc.sync.dma_start(out=c1w_t[:], in_=c1w_in[:, :])
            c2w_t = cpool.tile([64, 8], f32)
            nc.sync.dma_start(out=c2w_t[:], in_=c2w_in[:, :])

            def layernorm(pool, h_ap, rn, w_t, b_t, width):
                """in-place LN over the free dim of h_ap [rn, width]."""
                stats = pool.tile([P, 6], f32, tag="ln_stats")
                mv = pool.tile([P, 2], f32, tag="ln_mv")
                nc.vector.bn_stats(out=stats[:rn, :], in_=h_ap)
                nc.vector.bn_aggr(out=mv[:rn, :], in_=stats[:rn, :])
                nc.scalar.activation(out=mv[:rn, 1:2], in_=mv[:rn, 1:2],
                                     func=Act.Sqrt, bias=eps_t[:rn, :], scale=1.0)
                nc.vector.reciprocal(out=mv[:rn, 1:2], in_=mv[:rn, 1:2])
                nc.vector.tensor_scalar(out=h_ap, in0=h_ap,
                                        scalar1=mv[:rn, 0:1], scalar2=mv[:rn, 1:2],
                                        op0=AOp.subtract, op1=AOp.mult)
                nc.vector.tensor_tensor(out=h_ap, in0=h_ap, in1=w_t[:rn, :width],
                                        op=AOp.mult)
                nc.vector.tensor_tensor(out=h_ap, in0=h_ap, in1=b_t[:rn, :width],
                                        op=AOp.add)

            # ---------------- stage 0: LN + input projection + T1 ----------
            nb_run = DBG_BLOCKS if DBG_BLOCKS else N_BLOCKS
            with tc.tile_pool(name="s0", bufs=3) as s0, \
                 tc.tile_pool(name="s0ps", bufs=2, space="PSUM") as s0ps:
                for b in range(nb_run):
                    r0 = b * P
                    rn = min(P, NPC - r0)
                    xt = s0.tile([P, F_IN], f32, tag="xt")
                    nc.sync.dma_start(out=xt[:rn, :], in_=x_in[r0:r0 + rn, :])
                    layernorm(s0, xt[:rn, :], rn, pnw_t, pnb_t, F_IN)
                    # xp0 = LN(x) @ plW + plb  (transpose chunks, accumulate)
                    ps_t = s0ps.tile([P, P], f32, tag="s0tr")
                    xnT = s0.tile([P, 2, P], f32, tag="xnT")
                    for kk in range(2):
                        nc.tensor.transpose(out=ps_t[:, :rn],
                                            in_=xt[:rn, kk * P:(kk + 1) * P],
                                            identity=ident[:rn, :rn])
                        nc.vector.tensor_copy(out=xnT[:, kk, :rn], in_=ps_t[:, :rn])
                    ps_x = s0ps.tile([P, HID], f32, tag="s0mm")
                    for kk in range(2):
                        nc.tensor.matmul(out=ps_x[:rn, :], lhsT=xnT[:, kk, :rn],
                                         rhs=plw_t[:, kk, :],
                                         start=(kk == 0), stop=(kk == 1))
                    xp0 = s0.tile([P, HID], f32, tag="xp0")
                    nc.vector.tensor_tensor(out=xp0[:rn, :], in0=ps_x[:rn, :],
                                            in1=plb_t[:rn, :HID], op=AOp.add)
                    nc.sync.dma_start(out=xp0_own[r0:r0 + rn, :], in_=xp0[:rn, :])
                    # T1 = xp0 @ G1ext
                    nc.tensor.transpose(out=ps_t[:, :rn], in_=xp0[:rn, :],
                                        identity=ident[:rn, :rn])
                    xpT = s0.tile([P, P], f32, tag="xpT")
                    nc.vector.tensor_copy(out=xpT[:, :rn], in_=ps_t[:, :rn])
                    ps_p = s0ps.tile([P, TCOLS], f32, tag="s0pj")
                    nc.tensor.matmul(out=ps_p[:rn, :], lhsT=xpT[:, :rn],
                                     rhs=g1_t[:, :], start=True, stop=True)
                    t1t = s0.tile([P, TCOLS], f32, tag="t1t")
                    nc.vector.tensor_copy(out=t1t[:rn, :], in_=ps_p[:rn, :])
                    nc.sync.dma_start(out=t1_own[r0:r0 + rn, :], in_=t1t[:rn, :])

            if DBG_STAGE >= 1:
                nc.gpsimd.collective_compute(
                    "AllGather", AOp.bypass,
                    replica_groups=[list(range(N_CORES))],
                    ins=[t1_own[:, :]], outs=[t1_full[:, :]])
            if DBG_DUMP:
                nc.sync.dma_start(out=dbg_t1[:, :], in_=t1_own[:, :])
                if DBG_STAGE >= 1:
                    nc.sync.dma_start(out=dbg_t1f[:, :], in_=t1_full[:2 * P, :])

            # ---------------- GAT layers ----------------
            layers = () if DBG_STAGE < 2 else ((1,) if DBG_STAGE == 2 else (1, 2))
            for layer in layers:
                tbl = t1_full if layer == 1 else t2_full
                gb_t = g1b_t if layer == 1 else g2b_t
                nw_t = n1w_t if layer == 1 else n3w_t
                nb_t = n1b_t if layer == 1 else n3b_t
                resid = xp0_own if layer == 1 else xp1_own

                with tc.tile_pool(name=f"l{layer}", bufs=2) as wp, \
                     tc.tile_pool(name=f"l{layer}e", bufs=2) as ep, \
                     tc.tile_pool(name=f"l{layer}ps", bufs=2, space="PSUM") as pp, \
                     tc.tile_pool(name=f"l{layer}pe", bufs=1, space="PSUM") as pq:
                    off = int(kb[:DBG_B0].sum())
                    for b in range(DBG_B0, DBG_B0 + nb_run):
                        K = int(kb[b])
                        r0 = b * P
                        rn = min(P, NPC - r0)
                        idx_t = wp.tile([P, 3 * KMAX], i32, tag="idx")
                        nc.sync.dma_start(
                            out=idx_t[:, :3 * K],
                            in_=idx_in[:, 3 * off:3 * (off + K)])
                        gat = wp.tile([P, KMAX, TCOLS - NHEAD], f32, tag="gat")
                        ged = wp.tile([P, KMAX, NHEAD], f32, tag="ged")
                        # [P,1] offsets only: multi-column offset APs
                        # misaddress on hardware.
                        for k in range(K):
                            nc.gpsimd.indirect_dma_start(
                                out=gat[:, k, :], out_offset=None,
                                in_=tbl[:, :],
                                in_offset=bass.IndirectOffsetOnAxis(
                                    ap=idx_t[:, k:k + 1], axis=0))
                            nc.gpsimd.indirect_dma_start(
                                out=ged[:, k, :], out_offset=None,
                                in_=tbl[:, :],
                                in_offset=bass.IndirectOffsetOnAxis(
                                    ap=idx_t[:, K + k:K + k + 1], axis=0),
                                element_offset=HID + NHEAD)
                        # logits -> ex
                        lg = wp.tile([P, KMAX, NHEAD], f32, tag="lg")
                        nc.vector.tensor_tensor(
                            out=lg[:, :K, :], in0=gat[:, :K, HID:HID + NHEAD],
                            in1=ged[:, :K, :], op=AOp.add)
                        nc.vector.scalar_tensor_tensor(
                            out=lg[:, :K, :], in0=lg[:, :K, :], scalar=0.2,
                            in1=lg[:, :K, :], op0=AOp.mult, op1=AOp.max)
                        featx = wp.tile([P, KMAX, HID + NHEAD], f32, tag="featx")
                        nc.scalar.activation(out=featx[:, :K, HID:], in_=lg[:, :K, :],
                                             func=Act.Exp)
                        nc.vector.tensor_tensor(
                            out=featx[:, :K, 0:HID].rearrange(
                                "p k (h c) -> p k h c", c=CH),
                            in0=gat[:, :K, 0:HID].rearrange(
                                "p k (h c) -> p k h c", c=CH),
                            in1=featx[:, :K, HID:].unsqueeze(3).to_broadcast(
                                [P, K, NHEAD, CH]),
                            op=AOp.mult)
                        onht = wp.tile([P, KMAX, P], f32, tag="onht")
                        dlocf = idx_t[:, 2 * K:3 * K].bitcast(f32)
                        nc.vector.tensor_tensor(
                            out=onht[:, :K, :],
                            in0=dlocf.unsqueeze(2).to_broadcast([P, K, P]),
                            in1=iota_f[:, :].unsqueeze(1).to_broadcast([P, K, P]),
                            op=AOp.is_equal)
                        ps_a = pp.tile([P, HID + NHEAD], f32, tag="agg")
                        for k in range(K):
                            nc.tensor.matmul(out=ps_a[:, :], lhsT=onht[:, k, :],
                                             rhs=featx[:, k, :],
                                             start=(k == 0), stop=(k == K - 1))
                        # normalize, bias, relu, LN, residual
                        denr = ep.tile([P, NHEAD], f32, tag="denr")
                        nc.vector.tensor_scalar(
                            out=denr[:rn, :], in0=ps_a[:rn, HID:],
                            scalar1=1e-30, scalar2=None, op0=AOp.add)
                        nc.vector.reciprocal(out=denr[:rn, :], in_=denr[:rn, :])
                        h1 = ep.tile([P, HID], f32, tag="h1")
                        nc.vector.tensor_tensor(
                            out=h1[:rn, :].rearrange("p (h c) -> p h c", c=CH),
                            in0=ps_a[:rn, 0:HID].rearrange("p (h c) -> p h c", c=CH),
                            in1=denr[:rn, :].unsqueeze(2).to_broadcast(
                                [rn, NHEAD, CH]),
                            op=AOp.mult)
                        nc.vector.tensor_tensor(out=h1[:rn, :], in0=h1[:rn, :],
                                                in1=gb_t[:rn, :HID], op=AOp.add)
                        nc.scalar.activation(out=h1[:rn, :], in_=h1[:rn, :],
                                             func=Act.Relu)
                        layernorm(ep, h1[:rn, :], rn, nw_t, nb_t, HID)
                        xprev = ep.tile([P, HID], f32, tag="xprev")
                        nc.sync.dma_start(out=xprev[:rn, :],
                                          in_=resid[r0:r0 + rn, :])
                        xupd = ep.tile([P, HID], f32, tag="xupd")
                        nc.vector.tensor_tensor(out=xupd[:rn, :], in0=h1[:rn, :],
                                                in1=xprev[:rn, :], op=AOp.add)
                        if DBG_DUMP and layer == 1 and b == DBG_B0:
                            nc.sync.dma_start(out=dbg_gat[:, :, :],
                                              in_=gat[:, :, :])
                            nc.sync.dma_start(out=dbg_ged[:, :, :],
                                              in_=ged[:, :, :])
                            nc.sync.dma_start(out=dbg_h1[:rn, :], in_=h1[:rn, :])
                            psc = ep.tile([P, TCOLS - NHEAD], f32, tag="psc")
                            nc.vector.tensor_copy(out=psc[:, :], in_=ps_a[:, :])
                            nc.sync.dma_start(out=dbg_ps[:, :], in_=psc[:, :])
                        ps_t2 = pq.tile([P, P], f32, tag="tr")
                        if layer == 1:
                            nc.sync.dma_start(out=xp1_own[r0:r0 + rn, :],
                                              in_=xupd[:rn, :])
                            nc.tensor.transpose(out=ps_t2[:, :rn], in_=xupd[:rn, :],
                                                identity=ident[:rn, :rn])
                            xuT = ep.tile([P, P], f32, tag="xuT")
                            nc.vector.tensor_copy(out=xuT[:, :rn], in_=ps_t2[:, :rn])
                            ps_p2 = pq.tile([P, TCOLS], f32, tag="proj")
                            nc.tensor.matmul(out=ps_p2[:rn, :], lhsT=xuT[:, :rn],
                                             rhs=g2_t[:, :], start=True, stop=True)
                            t2t = ep.tile([P, TCOLS], f32, tag="t2t")
                            nc.vector.tensor_copy(out=t2t[:rn, :], in_=ps_p2[:rn, :])
                            nc.sync.dma_start(out=t2_own[r0:r0 + rn, :],
                                              in_=t2t[:rn, :])
                        else:
                            # final head: relu(xupd @ c1 + c1b) @ c2 + c2b
                            nc.tensor.transpose(out=ps_t2[:, :rn], in_=xupd[:rn, :],
                                                identity=ident[:rn, :rn])
                            xuT = ep.tile([P, P], f32, tag="xuT")
                            nc.vector.tensor_copy(out=xuT[:, :rn], in_=ps_t2[:, :rn])
                            ps_p2 = pq.tile([P, TCOLS], f32, tag="proj")
                            nc.tensor.matmul(out=ps_p2[:rn, :64], lhsT=xuT[:, :rn],
                                             rhs=c1w_t[:, :], start=True, stop=True)
                            hc = ep.tile([P, 64], f32, tag="hc")
                            nc.vector.tensor_tensor(out=hc[:rn, :],
                                                    in0=ps_p2[:rn, :64],
                                                    in1=c1b_t[:rn, :], op=AOp.add)
                            nc.scalar.activation(out=hc[:rn, :], in_=hc[:rn, :],
                                                 func=Act.Relu)
                            ps_t3 = pq.tile([64, P], f32, tag="tr2")
                            nc.tensor.transpose(out=ps_t3[:, :rn], in_=hc[:rn, :],
                                                identity=ident[:rn, :rn])
                            hcT = ep.tile([64, P], f32, tag="hcT")
                            nc.vector.tensor_copy(out=hcT[:, :rn], in_=ps_t3[:, :rn])
                            ps_o = pq.tile([P, 8], f32, tag="out")
                            nc.tensor.matmul(out=ps_o[:rn, :], lhsT=hcT[:, :rn],
                                             rhs=c2w_t[:, :], start=True, stop=True)
                            ot = ep.tile([P, 8], f32, tag="ot")
                            nc.vector.tensor_tensor(out=ot[:rn, :], in0=ps_o[:rn, :],
                                                    in1=c2b_t[:rn, :], op=AOp.add)
                            nc.sync.dma_start(out=out_ext[r0:r0 + rn, :],
                                              in_=ot[:rn, :])
                        off += K

                if layer == 1 and DBG_STAGE >= 3:
                    nc.gpsimd.collective_compute(
                        "AllGather", AOp.bypass,
                        replica_groups=[list(range(N_CORES))],
                        ins=[t2_own[:, :]], outs=[t2_full[:, :]])
    nc.finalize()
    return nc


def prep_inputs(inputs):
    idx_maps, kb = _prep_edges(np.asarray(inputs["ei_cites"]))
    g1ext = _head_fold(np.asarray(inputs["g1_W"], dtype=np.float32),
                       np.asarray(inputs["g1_as"], dtype=np.float32),
                       np.asarray(inputs["g1_ad"], dtype=np.float32))
    g2ext = _head_fold(np.asarray(inputs["g2_W"], dtype=np.float32),
                       np.asarray(inputs["g2_as"], dtype=np.float32),
                       np.asarray(inputs["g2_ad"], dtype=np.float32))
    plw = np.ascontiguousarray(
        np.asarray(inputs["pl_W"], dtype=np.float32)
        .reshape(2, P, HID).transpose(1, 0, 2))
    x_pat = np.asarray(inputs["x_patent"], dtype=np.float32)

    def f(k):
        return np.ascontiguousarray(np.asarray(inputs[k], dtype=np.float32))

    common = dict(plw=plw, pn_w=f("pn_w"), pn_b=f("pn_b"), pl_b=f("pl_b"),
                  g1ext=g1ext, g1_b=f("g1_b"), g2ext=g2ext, g2_b=f("g2_b"),
                  n1_w=f("n1_w"), n1_b=f("n1_b"), n3_w=f("n3_w"), n3_b=f("n3_b"),
                  c1w=f("c1_W"), c1_b=f("c1_b"), c2w=f("c2_W"), c2_b=f("c2_b"))
    in_maps = []
    for c in range(N_CORES):
        m = dict(common)
        m["x"] = np.ascontiguousarray(x_pat[c * NPC:(c + 1) * NPC])
        m["idx"] = idx_maps[c]
        in_maps.append(m)
    return in_maps, kb


def kernel_impl(inputs, **run_kwargs):
    from concourse.bass_utils import run_bass_kernel_spmd
    in_maps, kb = prep_inputs(inputs)
    nc = _build(kb)
    res = run_bass_kernel_spmd(nc, in_maps, core_ids=list(range(N_CORES)),
                               **run_kwargs)
    out = np.concatenate([r["out"] for r in res.results], axis=0)
    return out, res


def kernel(**inputs) -> np.ndarray:
    out, _ = kernel_impl(inputs)
    return out

